# revision 1
# baseline (speedup 1.0000x reference)
"""RNN-T JointNetwork kernel for 8x Trainium2 NeuronCores.

reference:
    enc_proj = einsum('btud,jd->btuj', enc_out, W_enc) + b_enc   # (B,T,1,J)
    dec_proj = einsum('btud,jd->btuj', dec_out, W_dec) + b_dec   # (B,1,U,J)
    joint    = tanh(enc_proj + dec_proj)                         # (B,T,U,J)
    out      = einsum('btuj,vj->btuv', joint, W_out) + b_out     # (B,T,U,V)

Strategy: data-parallel over batch B=8 across the 8 cores (one b each).
Per core:
  - enc_projT [J, T] and dec_projT [J, U] via small GEMMs (weights stationary,
    host-pretransposed operands), bias_joint = b_enc+b_dec folded into dec_projT.
  - loop over 32 f-chunks (f = t*U+u, 8 t-values x 64 u = 512 f per chunk):
      jointT[j, f] = tanh(enc_projT[j,t] + dec_projT[j,u])  (DVE bcast-add + ACT tanh)
      out[f, v]    = jointT.T @ W_outT   (float32r matmuls, PSUM accum over 5 j-tiles)
      += b_out (DVE add with host-replicated bias tile) and contiguous DMA out.
All matmuls use float32r (TF32-like: full-rate streaming, fp32 accumulate).
"""

import sys

import numpy as np

if "/opt/trn_rl_repo" not in sys.path:
    sys.path.insert(0, "/opt/trn_rl_repo")

B, T, U = 8, 256, 64
D, J, V = 512, 640, 1024
P = 128
ND, NJ, NV = D // P, J // P, V // P  # 4, 5, 8
TCH = 8  # t-values per f-chunk
NCHUNK = T // TCH  # 32
FCH = TCH * U  # 512 f-positions per chunk
NFT = FCH // P  # 4 f-tiles per chunk

_prog_cache = {}


def build_program():
    import concourse.tile as tile
    from concourse import bacc, mybir

    f32 = mybir.dt.float32
    f32r = mybir.dt.float32r
    Tanh = mybir.ActivationFunctionType.Tanh
    Ident = mybir.ActivationFunctionType.Identity

    nc = bacc.Bacc("TRN2", target_bir_lowering=False, debug=False)

    enc_T = nc.dram_tensor("enc_T", [D, T], f32, kind="ExternalInput").ap()
    dec_T = nc.dram_tensor("dec_T", [D, U], f32, kind="ExternalInput").ap()
    w_enc_T = nc.dram_tensor("w_enc_T", [D, J], f32, kind="ExternalInput").ap()
    w_dec_T = nc.dram_tensor("w_dec_T", [D, J], f32, kind="ExternalInput").ap()
    w_out_T = nc.dram_tensor("w_out_T", [J, V], f32, kind="ExternalInput").ap()
    bias_j = nc.dram_tensor("bias_j", [J, 1], f32, kind="ExternalInput").ap()
    b_out_rep = nc.dram_tensor("b_out_rep", [P, V], f32, kind="ExternalInput").ap()
    out = nc.dram_tensor("out", [T * U, V], f32, kind="ExternalOutput").ap()

    with tile.TileContext(nc) as tc:
        with (
            tc.tile_pool(name="const", bufs=1) as constp,
            tc.tile_pool(name="proj", bufs=1) as projp,
            tc.tile_pool(name="pre", bufs=6) as prep,
            tc.tile_pool(name="joint", bufs=10) as jointp,
            tc.tile_pool(name="osb", bufs=8) as osbp,
            tc.tile_pool(name="ps", bufs=4, space="PSUM") as psp,
        ):
            # ---- load weights / inputs (one-time) ----
            # f32r matmul operands must be written by a rounding producer
            # (BIR verifier) — stage DMA loads in f32 then round-copy to f32r.
            def load_round(shape, dram_ap, tag):
                stg = constp.tile(shape, f32, tag=f"stage_{tag}")
                nc.sync.dma_start(out=stg[:], in_=dram_ap)
                t_ = constp.tile(shape, f32r, tag=tag)
                nc.vector.tensor_copy(t_[:], stg[:])
                return t_

            w_out_sb = [
                load_round([P, V], w_out_T[jt * P : (jt + 1) * P, :], f"wout{jt}")
                for jt in range(NJ)
            ]
            enc_sb, dec_sb, wenc_sb, wdec_sb = [], [], [], []
            for dt_ in range(ND):
                sl = slice(dt_ * P, (dt_ + 1) * P)
                enc_sb.append(load_round([P, T], enc_T[sl, :], f"enc{dt_}"))
                dec_sb.append(load_round([P, U], dec_T[sl, :], f"dec{dt_}"))
                wenc_sb.append(load_round([P, J], w_enc_T[sl, :], f"wenc{dt_}"))
                wdec_sb.append(load_round([P, J], w_dec_T[sl, :], f"wdec{dt_}"))
            bj_sb = constp.tile([P, NJ], f32, tag="bj")
            nc.sync.dma_start(
                out=bj_sb[:],
                in_=bias_j.rearrange("(jt p) one -> p (jt one)", p=P),
            )
            b_out_sb = constp.tile([P, V], f32, tag="bout")
            nc.sync.dma_start(out=b_out_sb[:], in_=b_out_rep[:, :])

            # ---- projections: enc_projT [J, T], dec_projT [J, U] ----
            enc_proj, dec_proj = [], []
            for jt in range(NJ):
                ps = psp.tile([P, V], f32, tag="ps")
                for dt_ in range(ND):
                    nc.tensor.matmul(
                        ps[:, :T],
                        lhsT=wenc_sb[dt_][:, jt * P : (jt + 1) * P],
                        rhs=enc_sb[dt_][:],
                        start=(dt_ == 0),
                        stop=(dt_ == ND - 1),
                    )
                t_ = projp.tile([P, T], f32, tag=f"encproj{jt}")
                nc.scalar.copy(t_[:], ps[:, :T])
                enc_proj.append(t_)
            for jt in range(NJ):
                ps = psp.tile([P, V], f32, tag="ps")
                for dt_ in range(ND):
                    nc.tensor.matmul(
                        ps[:, :U],
                        lhsT=wdec_sb[dt_][:, jt * P : (jt + 1) * P],
                        rhs=dec_sb[dt_][:],
                        start=(dt_ == 0),
                        stop=(dt_ == ND - 1),
                    )
                t_ = projp.tile([P, U], f32, tag=f"decproj{jt}")
                nc.scalar.activation(t_[:], ps[:, :U], Ident, bias=bj_sb[:, jt : jt + 1])
                dec_proj.append(t_)

            # ---- main loop over f-chunks ----
            for ch in range(NCHUNK):
                jts = []
                for jt in range(NJ):
                    pre = prep.tile([P, FCH], f32, tag="pre")
                    enc_b = (
                        enc_proj[jt][:, ch * TCH : (ch + 1) * TCH]
                        .unsqueeze(2)
                        .broadcast_to([P, TCH, U])
                    )
                    dec_b = dec_proj[jt][:].unsqueeze(1).broadcast_to([P, TCH, U])
                    nc.vector.tensor_add(
                        pre[:].rearrange("p (t u) -> p t u", t=TCH), enc_b, dec_b
                    )
                    jtl = jointp.tile([P, FCH], f32r, tag="joint")
                    nc.scalar.activation(jtl[:], pre[:], Tanh)
                    jts.append(jtl)
                for ft in range(NFT):
                    ps = psp.tile([P, V], f32, tag="ps")
                    for vh in range(V // 512):
                        for jt in range(NJ):
                            nc.tensor.matmul(
                                ps[:, vh * 512 : (vh + 1) * 512],
                                lhsT=jts[jt][:, ft * P : (ft + 1) * P],
                                rhs=w_out_sb[jt][:, vh * 512 : (vh + 1) * 512],
                                start=(jt == 0),
                                stop=(jt == NJ - 1),
                            )
                    o = osbp.tile([P, V], f32, tag="osb")
                    nc.vector.tensor_add(o[:], ps[:], b_out_sb[:])
                    f0 = ch * FCH + ft * P
                    nc.sync.dma_start(out=out[f0 : f0 + P, :], in_=o[:])
    nc.compile()
    return nc


def _get_program():
    if "nc" not in _prog_cache:
        _prog_cache["nc"] = build_program()
    return _prog_cache["nc"]


def make_in_maps(inputs):
    enc_out = np.asarray(inputs["enc_out"], dtype=np.float32)  # (B, T, 1, D)
    dec_out = np.asarray(inputs["dec_out"], dtype=np.float32)  # (B, 1, U, D)
    W_enc = np.asarray(inputs["W_enc"], dtype=np.float32)  # (J, D)
    b_enc = np.asarray(inputs["b_enc"], dtype=np.float32)
    W_dec = np.asarray(inputs["W_dec"], dtype=np.float32)
    b_dec = np.asarray(inputs["b_dec"], dtype=np.float32)
    W_out = np.asarray(inputs["W_out"], dtype=np.float32)  # (V, J)
    b_out = np.asarray(inputs["b_out"], dtype=np.float32)

    w_enc_T = np.ascontiguousarray(W_enc.T)  # [D, J]
    w_dec_T = np.ascontiguousarray(W_dec.T)  # [D, J]
    w_out_T = np.ascontiguousarray(W_out.T)  # [J, V]
    bias_j = np.ascontiguousarray((b_enc + b_dec).reshape(J, 1))
    b_out_rep = np.ascontiguousarray(np.broadcast_to(b_out, (P, V)))

    in_maps = []
    for b in range(B):
        in_maps.append(
            {
                "enc_T": np.ascontiguousarray(enc_out[b, :, 0, :].T),  # [D, T]
                "dec_T": np.ascontiguousarray(dec_out[b, 0, :, :].T),  # [D, U]
                "w_enc_T": w_enc_T,
                "w_dec_T": w_dec_T,
                "w_out_T": w_out_T,
                "bias_j": bias_j,
                "b_out_rep": b_out_rep,
            }
        )
    return in_maps


def kernel(**inputs):
    from concourse.bass_utils import run_bass_kernel_spmd

    nc = _get_program()
    in_maps = make_in_maps(inputs)
    res = run_bass_kernel_spmd(nc, in_maps, list(range(B)))
    outs = [res.results[i]["out"].reshape(T, U, V) for i in range(B)]
    return np.stack(outs, axis=0)



# revision 10
# speedup vs baseline: 243.9617x; 243.9617x over previous
"""RNN-T JointNetwork kernel for 8x Trainium2 NeuronCores.

reference:
    enc_proj = einsum('btud,jd->btuj', enc_out, W_enc) + b_enc   # (B,T,1,J)
    dec_proj = einsum('btud,jd->btuj', dec_out, W_dec) + b_dec   # (B,1,U,J)
    joint    = tanh(enc_proj + dec_proj)                         # (B,T,U,J)
    out      = einsum('btuj,vj->btuv', joint, W_out) + b_out     # (B,T,U,V)

Strategy: data-parallel over batch B=8 across the 8 cores (one b each).
Per core (all GEMM operands bf16, fp32 PSUM accumulate; rel err ~3e-3):
  - inputs host-packed partition-major so each tensor is ONE dma_start with
    2-10KB contiguous rows per partition; loads issued from 3 different
    engines (sync/scalar/vector) so the sequencers don't serialize them.
  - enc_projT [J, T] and dec_projT [J, U] via small GEMMs, joint bias folded
    into dec_projT's PSUM drain.
  - software-pipelined loop over 32 f-chunks (f = t*U+u, 8 t x 64 u = 512 f):
    joint tiles for chunk ch+1 are built while chunk ch's matmuls run, and
    BEFORE chunk ch's drains are emitted, so the in-order DVE stream never
    blocks the next chunk's inputs behind the previous chunk's drains.
      jointT[j,f] = tanh(enc_projT[j,t] + dec_projT[j,u]):
        jt 0,1: DVE broadcast-add -> ACT tanh      (bf16 out)
        jt 2:   Pool broadcast-add -> ACT tanh
        jt 3,4: fused on ACT: tanh(dec_proj + bias=enc_col), one inst per t
      out[f,v] = jointT.T @ W_outT  (bf16 matmuls, PSUM accum over 5 j-tiles)
      += b_out (DVE add from PSUM), contiguous DMA out.
"""

import sys

import numpy as np

if "/opt/trn_rl_repo" not in sys.path:
    sys.path.insert(0, "/opt/trn_rl_repo")

B, T, U = 8, 256, 64
D, J, V = 512, 640, 1024
P = 128
ND, NJ = D // P, J // P  # 4, 5
TCH = 8  # t-values per f-chunk
NCHUNK = T // TCH  # 32
FCH = TCH * U  # 512 f-positions per chunk
NFT = FCH // P  # 4 f-tiles per chunk

_prog_cache = {}


def build_program():
    import concourse.tile as tile
    from concourse import bacc, mybir

    f32 = mybir.dt.float32
    bf16 = mybir.dt.bfloat16
    Tanh = mybir.ActivationFunctionType.Tanh
    Ident = mybir.ActivationFunctionType.Identity

    nc = bacc.Bacc("TRN2", target_bir_lowering=False, debug=False)

    # Host-packed, partition-major: row p holds that partition's data for
    # every (sub-tile, column) pair, so one DMA covers the whole tensor.
    enc_pk = nc.dram_tensor("enc_pk", [P, ND * T], bf16, kind="ExternalInput").ap()
    dec_pk = nc.dram_tensor("dec_pk", [P, ND * U], bf16, kind="ExternalInput").ap()
    wenc_pk = nc.dram_tensor("wenc_pk", [P, ND * J], bf16, kind="ExternalInput").ap()
    wdec_pk = nc.dram_tensor("wdec_pk", [P, ND * J], bf16, kind="ExternalInput").ap()
    wout_pk = nc.dram_tensor("wout_pk", [P, NJ * V], bf16, kind="ExternalInput").ap()
    bias_pk = nc.dram_tensor("bias_pk", [P, NJ], f32, kind="ExternalInput").ap()
    b_out_rep = nc.dram_tensor("b_out_rep", [P, V], f32, kind="ExternalInput").ap()
    out = nc.dram_tensor("out", [T * U, V], f32, kind="ExternalOutput").ap()

    with tile.TileContext(nc) as tc:
        with (
            tc.tile_pool(name="const", bufs=1) as constp,
            tc.tile_pool(name="proj", bufs=1) as projp,
            tc.tile_pool(name="pre", bufs=6) as prep,
            tc.tile_pool(name="joint", bufs=10) as jointp,
            tc.tile_pool(name="osb", bufs=8) as osbp,
            tc.tile_pool(name="ps", bufs=3, space="PSUM") as psp,
            tc.tile_pool(name="pproj", bufs=2, space="PSUM") as pprojp,
        ):
            # ---- input loads: one DMA per tensor, 3 issue engines ----
            def load(shape, dt_, dram_ap, tag, eng):
                t_ = constp.tile(shape, dt_, tag=tag)
                eng.dma_start(out=t_[:], in_=dram_ap)
                return t_

            # sync: what the enc projection needs first
            wenc_sb = load([P, ND * J], bf16, wenc_pk[:, :], "wenc", nc.sync)
            enc_sb = load([P, ND * T], bf16, enc_pk[:, :], "enc", nc.sync)
            # scalar: dec projection inputs + joint bias
            wdec_sb = load([P, ND * J], bf16, wdec_pk[:, :], "wdec", nc.scalar)
            dec_sb = load([P, ND * U], bf16, dec_pk[:, :], "dec", nc.scalar)
            bj_sb = load([P, NJ], f32, bias_pk[:, :], "bj", nc.scalar)
            # gpsimd (SWDGE): main-GEMM weights + output bias
            w_out_sb = load([P, NJ * V], bf16, wout_pk[:, :], "wout", nc.gpsimd)
            b_out_sb = load([P, V], f32, b_out_rep[:, :], "bout", nc.gpsimd)

            # ---- projections: enc_projT [J, T] f32, dec_projT [J, U] f32 ----
            enc_proj, dec_proj = [], []
            for jt in range(NJ):
                ps = pprojp.tile([P, 512], f32, tag="psp")
                for dt_ in range(ND):
                    nc.tensor.matmul(
                        ps[:, :T],
                        lhsT=wenc_sb[:, dt_ * J + jt * P : dt_ * J + (jt + 1) * P],
                        rhs=enc_sb[:, dt_ * T : (dt_ + 1) * T],
                        start=(dt_ == 0),
                        stop=(dt_ == ND - 1),
                    )
                t_ = projp.tile([P, T], f32, tag=f"encproj{jt}")
                nc.scalar.copy(t_[:], ps[:, :T])
                enc_proj.append(t_)
            for jt in range(NJ):
                ps = pprojp.tile([P, 512], f32, tag="psp")
                for dt_ in range(ND):
                    nc.tensor.matmul(
                        ps[:, :U],
                        lhsT=wdec_sb[:, dt_ * J + jt * P : dt_ * J + (jt + 1) * P],
                        rhs=dec_sb[:, dt_ * U : (dt_ + 1) * U],
                        start=(dt_ == 0),
                        stop=(dt_ == ND - 1),
                    )
                t_ = projp.tile([P, U], f32, tag=f"decproj{jt}")
                nc.scalar.activation(t_[:], ps[:, :U], Ident, bias=bj_sb[:, jt : jt + 1])
                dec_proj.append(t_)

            # ---- software-pipelined main loop over f-chunks ----
            def build_jts(ch):
                jts = []
                for jt in range(NJ):
                    jtl = jointp.tile([P, FCH], bf16, tag="joint")
                    if jt >= 3:
                        for tl in range(TCH):
                            t = ch * TCH + tl
                            nc.scalar.activation(
                                jtl[:, tl * U : (tl + 1) * U],
                                dec_proj[jt][:],
                                Tanh,
                                bias=enc_proj[jt][:, t : t + 1],
                            )
                    else:
                        pre = prep.tile([P, FCH], f32, tag="pre")
                        enc_b = (
                            enc_proj[jt][:, ch * TCH : (ch + 1) * TCH]
                            .unsqueeze(2)
                            .broadcast_to([P, TCH, U])
                        )
                        dec_b = dec_proj[jt][:].unsqueeze(1).broadcast_to([P, TCH, U])
                        eng = nc.gpsimd if jt == 2 else nc.vector
                        eng.tensor_add(
                            pre[:].rearrange("p (t u) -> p t u", t=TCH), enc_b, dec_b
                        )
                        nc.scalar.activation(jtl[:], pre[:], Tanh)
                    jts.append(jtl)
                return jts

            jts_cur = build_jts(0)
            for ch in range(NCHUNK):
                jts_next = build_jts(ch + 1) if ch + 1 < NCHUNK else None
                for ft in range(NFT):
                    ps = psp.tile([P, V], f32, tag="ps")
                    for vh in range(V // 512):
                        for jt in range(NJ):
                            nc.tensor.matmul(
                                ps[:, vh * 512 : (vh + 1) * 512],
                                lhsT=jts_cur[jt][:, ft * P : (ft + 1) * P],
                                rhs=w_out_sb[
                                    :, jt * V + vh * 512 : jt * V + (vh + 1) * 512
                                ],
                                start=(jt == 0),
                                stop=(jt == NJ - 1),
                            )
                    o = osbp.tile([P, V], f32, tag="osb")
                    nc.vector.tensor_add(o[:], ps[:], b_out_sb[:])
                    f0 = ch * FCH + ft * P
                    nc.sync.dma_start(out=out[f0 : f0 + P, :], in_=o[:])
                jts_cur = jts_next
    nc.compile()
    return nc


def _get_program():
    if "nc" not in _prog_cache:
        _prog_cache["nc"] = build_program()
    return _prog_cache["nc"]


def _pack(a, ntiles):
    """[ntiles*P, C] -> [P, ntiles*C] partition-major packing."""
    C = a.shape[1]
    return np.ascontiguousarray(
        a.reshape(ntiles, P, C).transpose(1, 0, 2).reshape(P, ntiles * C)
    )


def make_in_maps(inputs):
    import ml_dtypes

    bf16 = ml_dtypes.bfloat16

    enc_out = np.asarray(inputs["enc_out"], dtype=np.float32)  # (B, T, 1, D)
    dec_out = np.asarray(inputs["dec_out"], dtype=np.float32)  # (B, 1, U, D)
    W_enc = np.asarray(inputs["W_enc"], dtype=np.float32)  # (J, D)
    b_enc = np.asarray(inputs["b_enc"], dtype=np.float32)
    W_dec = np.asarray(inputs["W_dec"], dtype=np.float32)
    b_dec = np.asarray(inputs["b_dec"], dtype=np.float32)
    W_out = np.asarray(inputs["W_out"], dtype=np.float32)  # (V, J)
    b_out = np.asarray(inputs["b_out"], dtype=np.float32)

    wenc_pk = _pack(W_enc.T.astype(bf16), ND)  # [P, ND*J]
    wdec_pk = _pack(W_dec.T.astype(bf16), ND)
    wout_pk = _pack(W_out.T.astype(bf16), NJ)  # [P, NJ*V]
    bias_pk = np.ascontiguousarray((b_enc + b_dec).reshape(NJ, P).T)  # [P, NJ]
    b_out_rep = np.ascontiguousarray(np.broadcast_to(b_out, (P, V)))

    in_maps = []
    for b in range(B):
        in_maps.append(
            {
                "enc_pk": _pack(enc_out[b, :, 0, :].T.astype(bf16), ND),
                "dec_pk": _pack(dec_out[b, 0, :, :].T.astype(bf16), ND),
                "wenc_pk": wenc_pk,
                "wdec_pk": wdec_pk,
                "wout_pk": wout_pk,
                "bias_pk": bias_pk,
                "b_out_rep": b_out_rep,
            }
        )
    return in_maps


def kernel(**inputs):
    from concourse.bass_utils import run_bass_kernel_spmd

    nc = _get_program()
    in_maps = make_in_maps(inputs)
    res = run_bass_kernel_spmd(nc, in_maps, list(range(B)))
    outs = [res.results[i]["out"].reshape(T, U, V) for i in range(B)]
    return np.stack(outs, axis=0)


# revision 14
# speedup vs baseline: 246.6553x; 1.0110x over previous
"""RNN-T JointNetwork kernel for 8x Trainium2 NeuronCores.

reference:
    enc_proj = einsum('btud,jd->btuj', enc_out, W_enc) + b_enc   # (B,T,1,J)
    dec_proj = einsum('btud,jd->btuj', dec_out, W_dec) + b_dec   # (B,1,U,J)
    joint    = tanh(enc_proj + dec_proj)                         # (B,T,U,J)
    out      = einsum('btuj,vj->btuv', joint, W_out) + b_out     # (B,T,U,V)

Strategy: data-parallel over batch B=8 across the 8 cores (one b each).
Per core (all GEMM operands bf16, fp32 PSUM accumulate; rel err ~3e-3):
  - inputs host-packed partition-major so each tensor is ONE dma_start with
    2-10KB contiguous rows per partition; loads issued from 3 different
    engines (sync/scalar/vector) so the sequencers don't serialize them.
  - enc_projT [J, T] and dec_projT [J, U] via small GEMMs, joint bias folded
    into dec_projT's PSUM drain.
  - software-pipelined loop over 32 f-chunks (f = t*U+u, 8 t x 64 u = 512 f):
    joint tiles for chunk ch+1 are built while chunk ch's matmuls run, and
    BEFORE chunk ch's drains are emitted, so the in-order DVE stream never
    blocks the next chunk's inputs behind the previous chunk's drains.
      jointT[j,f] = tanh(enc_projT[j,t] + dec_projT[j,u]):
        jt 0,1: DVE broadcast-add -> ACT tanh      (bf16 out)
        jt 2:   Pool broadcast-add -> ACT tanh
        jt 3,4: fused on ACT: tanh(dec_proj + bias=enc_col), one inst per t
      out[f,v] = jointT.T @ W_outT  (bf16 matmuls, PSUM accum over 5 j-tiles)
      += b_out (DVE add from PSUM), contiguous DMA out.
"""

import sys

import numpy as np

if "/opt/trn_rl_repo" not in sys.path:
    sys.path.insert(0, "/opt/trn_rl_repo")

B, T, U = 8, 256, 64
D, J, V = 512, 640, 1024
P = 128
ND, NJ = D // P, J // P  # 4, 5
TCH = 8  # t-values per f-chunk
NCHUNK = T // TCH  # 32
FCH = TCH * U  # 512 f-positions per chunk
NFT = FCH // P  # 4 f-tiles per chunk

_prog_cache = {}


def build_program():
    import concourse.tile as tile
    from concourse import bacc, mybir

    f32 = mybir.dt.float32
    bf16 = mybir.dt.bfloat16
    Tanh = mybir.ActivationFunctionType.Tanh
    Ident = mybir.ActivationFunctionType.Identity

    nc = bacc.Bacc("TRN2", target_bir_lowering=False, debug=False)

    # Host-packed, partition-major: row p holds that partition's data for
    # every (sub-tile, column) pair, so one DMA covers the whole tensor.
    enc_pk = nc.dram_tensor("enc_pk", [P, ND * T], bf16, kind="ExternalInput").ap()
    dec_pk = nc.dram_tensor("dec_pk", [P, ND * U], bf16, kind="ExternalInput").ap()
    wenc_pk = nc.dram_tensor("wenc_pk", [P, ND * J], bf16, kind="ExternalInput").ap()
    wdec_pk = nc.dram_tensor("wdec_pk", [P, ND * J], bf16, kind="ExternalInput").ap()
    wout_pk = nc.dram_tensor("wout_pk", [P, NJ * V], bf16, kind="ExternalInput").ap()
    bias_pk = nc.dram_tensor("bias_pk", [P, NJ], f32, kind="ExternalInput").ap()
    b_out_rep = nc.dram_tensor("b_out_rep", [P, V], f32, kind="ExternalInput").ap()
    out = nc.dram_tensor("out", [T * U, V], f32, kind="ExternalOutput").ap()

    with tile.TileContext(nc) as tc:
        with (
            tc.tile_pool(name="const", bufs=1) as constp,
            tc.tile_pool(name="proj", bufs=1) as projp,
            tc.tile_pool(name="pre", bufs=9) as prep,
            tc.tile_pool(name="joint", bufs=15) as jointp,
            tc.tile_pool(name="osb", bufs=8) as osbp,
            tc.tile_pool(name="ps", bufs=4, space="PSUM") as psp,
        ):
            # ---- input loads ----
            # Stripe every tensor across the 3 DMA-issuing queues
            # (sync/scalar HWDGE + gpsimd SWDGE) by partition range. The
            # hardware-dynamic DMA engines serve all queues round-robin, so
            # same-priority stripes drain together at full aggregate rate and
            # tensors complete in program order: projection inputs first, then
            # W_out, then the output bias (first needed at the first drain).
            dma_engs = None

            def load(shape, dt_, dram_ap, tag):
                t_ = constp.tile(shape, dt_, tag=tag)
                ncols = shape[1]
                step = (ncols // 3 + 15) // 16 * 16
                c0 = 0
                for eng in dma_engs:
                    c1 = min(c0 + step, ncols)
                    if c1 > c0:
                        eng.dma_start(out=t_[:, c0:c1], in_=dram_ap[:, c0:c1])
                    c0 = c1
                return t_

            dma_engs = (nc.sync, nc.scalar, nc.gpsimd)
            enc_sb = load([P, ND * T], bf16, enc_pk, "enc")
            wenc_sb = load([P, ND * J], bf16, wenc_pk, "wenc")
            dec_sb = load([P, ND * U], bf16, dec_pk, "dec")
            wdec_sb = load([P, ND * J], bf16, wdec_pk, "wdec")
            bj_sb = constp.tile([P, NJ], f32, tag="bj")
            nc.sync.dma_start(out=bj_sb[:], in_=bias_pk[:, :])
            w_out_sb = load([P, NJ * V], bf16, wout_pk, "wout")
            b_out_sb = load([P, V], f32, b_out_rep, "bout")

            # ---- projections: enc_projT [J, T] f32, dec_projT [J, U] f32 ----
            enc_proj, dec_proj = [], []
            for jt in range(NJ):
                ps = psp.tile([P, V], f32, tag="ps")
                for dt_ in range(ND):
                    nc.tensor.matmul(
                        ps[:, :T],
                        lhsT=wenc_sb[:, dt_ * J + jt * P : dt_ * J + (jt + 1) * P],
                        rhs=enc_sb[:, dt_ * T : (dt_ + 1) * T],
                        start=(dt_ == 0),
                        stop=(dt_ == ND - 1),
                    )
                t_ = projp.tile([P, T], f32, tag=f"encproj{jt}")
                nc.scalar.copy(t_[:], ps[:, :T])
                enc_proj.append(t_)
            for jt in range(NJ):
                ps = psp.tile([P, V], f32, tag="ps")
                for dt_ in range(ND):
                    nc.tensor.matmul(
                        ps[:, :U],
                        lhsT=wdec_sb[:, dt_ * J + jt * P : dt_ * J + (jt + 1) * P],
                        rhs=dec_sb[:, dt_ * U : (dt_ + 1) * U],
                        start=(dt_ == 0),
                        stop=(dt_ == ND - 1),
                    )
                t_ = projp.tile([P, U], f32, tag=f"decproj{jt}")
                nc.scalar.activation(t_[:], ps[:, :U], Ident, bias=bj_sb[:, jt : jt + 1])
                dec_proj.append(t_)

            # ---- software-pipelined main loop over f-chunks ----
            def build_jts(ch):
                jts = []
                for jt in range(NJ):
                    jtl = jointp.tile([P, FCH], bf16, tag="joint")
                    if jt >= 3:
                        for tl in range(TCH):
                            t = ch * TCH + tl
                            nc.scalar.activation(
                                jtl[:, tl * U : (tl + 1) * U],
                                dec_proj[jt][:],
                                Tanh,
                                bias=enc_proj[jt][:, t : t + 1],
                            )
                    else:
                        pre = prep.tile([P, FCH], f32, tag="pre")
                        enc_b = (
                            enc_proj[jt][:, ch * TCH : (ch + 1) * TCH]
                            .unsqueeze(2)
                            .broadcast_to([P, TCH, U])
                        )
                        dec_b = dec_proj[jt][:].unsqueeze(1).broadcast_to([P, TCH, U])
                        eng = nc.gpsimd if jt == 2 else nc.vector
                        eng.tensor_add(
                            pre[:].rearrange("p (t u) -> p t u", t=TCH), enc_b, dec_b
                        )
                        nc.scalar.activation(jtl[:], pre[:], Tanh)
                    jts.append(jtl)
                return jts

            jts_cur = build_jts(0)
            for ch in range(NCHUNK):
                jts_next = build_jts(ch + 1) if ch + 1 < NCHUNK else None
                for ft in range(NFT):
                    ps = psp.tile([P, V], f32, tag="ps")
                    for vh in range(V // 512):
                        for jt in range(NJ):
                            nc.tensor.matmul(
                                ps[:, vh * 512 : (vh + 1) * 512],
                                lhsT=jts_cur[jt][:, ft * P : (ft + 1) * P],
                                rhs=w_out_sb[
                                    :, jt * V + vh * 512 : jt * V + (vh + 1) * 512
                                ],
                                start=(jt == 0),
                                stop=(jt == NJ - 1),
                            )
                    o = osbp.tile([P, V], f32, tag="osb")
                    f0 = ch * FCH + ft * P
                    if ch == NCHUNK - 1:
                        # drain tail: flush per 512-col half so the last DMA
                        # starts as soon as its half of the PSUM is complete
                        for vh in range(V // 512):
                            sl = slice(vh * 512, (vh + 1) * 512)
                            nc.vector.tensor_add(o[:, sl], ps[:, sl], b_out_sb[:, sl])
                            nc.sync.dma_start(out=out[f0 : f0 + P, sl], in_=o[:, sl])
                    else:
                        nc.vector.tensor_add(o[:], ps[:], b_out_sb[:])
                        nc.sync.dma_start(out=out[f0 : f0 + P, :], in_=o[:])
                jts_cur = jts_next
    nc.compile()
    return nc


def _get_program():
    if "nc" not in _prog_cache:
        _prog_cache["nc"] = build_program()
    return _prog_cache["nc"]


def _pack(a, ntiles):
    """[ntiles*P, C] -> [P, ntiles*C] partition-major packing."""
    C = a.shape[1]
    return np.ascontiguousarray(
        a.reshape(ntiles, P, C).transpose(1, 0, 2).reshape(P, ntiles * C)
    )


def make_in_maps(inputs):
    import ml_dtypes

    bf16 = ml_dtypes.bfloat16

    enc_out = np.asarray(inputs["enc_out"], dtype=np.float32)  # (B, T, 1, D)
    dec_out = np.asarray(inputs["dec_out"], dtype=np.float32)  # (B, 1, U, D)
    W_enc = np.asarray(inputs["W_enc"], dtype=np.float32)  # (J, D)
    b_enc = np.asarray(inputs["b_enc"], dtype=np.float32)
    W_dec = np.asarray(inputs["W_dec"], dtype=np.float32)
    b_dec = np.asarray(inputs["b_dec"], dtype=np.float32)
    W_out = np.asarray(inputs["W_out"], dtype=np.float32)  # (V, J)
    b_out = np.asarray(inputs["b_out"], dtype=np.float32)

    wenc_pk = _pack(W_enc.T.astype(bf16), ND)  # [P, ND*J]
    wdec_pk = _pack(W_dec.T.astype(bf16), ND)
    wout_pk = _pack(W_out.T.astype(bf16), NJ)  # [P, NJ*V]
    bias_pk = np.ascontiguousarray((b_enc + b_dec).reshape(NJ, P).T)  # [P, NJ]
    b_out_rep = np.ascontiguousarray(np.broadcast_to(b_out, (P, V)))

    in_maps = []
    for b in range(B):
        in_maps.append(
            {
                "enc_pk": _pack(enc_out[b, :, 0, :].T.astype(bf16), ND),
                "dec_pk": _pack(dec_out[b, 0, :, :].T.astype(bf16), ND),
                "wenc_pk": wenc_pk,
                "wdec_pk": wdec_pk,
                "wout_pk": wout_pk,
                "bias_pk": bias_pk,
                "b_out_rep": b_out_rep,
            }
        )
    return in_maps


def kernel(**inputs):
    from concourse.bass_utils import run_bass_kernel_spmd

    nc = _get_program()
    in_maps = make_in_maps(inputs)
    res = run_bass_kernel_spmd(nc, in_maps, list(range(B)))
    outs = [res.results[i]["out"].reshape(T, U, V) for i in range(B)]
    return np.stack(outs, axis=0)
